# revision 32
# baseline (speedup 1.0000x reference)
"""GroupedESN Trainium2 kernel.

Problem: E=8 echo-state networks, batch B=16, T=512 steps, reservoir R=512,
input D=8.  h_{t+1} = (1-a) h_t + a tanh(W_in x_t + W_res h_t), output is the
final state concatenated over ESNs -> [B, E*R].

Sharding: one ESN per NeuronCore (8 cores).  Inside a core the recurrence is
sequential over T; per step the tensor engine re-ingests W (fp16 stationary,
fast-weight-load) as 16 [128,128] chunks.

State substitution (a folded into W, so per-core program is data-independent):
  g = h / a,  W'' = a * W_res,  c = 1 - a
  g_{t+1} = c g_t + tanh(u_t + W'' g_t)
Split g = sigma + tau so the only serial op between steps is the tanh:
  tau_{t+1}   = tanh(u_t + W'' sigma_t + W'' tau_t)     (scalar engine)
  sigma_{t+1} = c (sigma_t + tau_t)                     (vector, off-chain)

PSUM layout: 8 banks = (block parity) x (rc half) x (step parity).  Input
projections u_t are matmul'd directly into the banks (start=True), recurrence
matmuls accumulate on top (start=False), tanh reads PSUM.
"""

import os
import sys

import numpy as np

for _p in ("/opt/trn_rl_repo", "/root/.axon_site/_ro/trn_rl_repo"):
    if _p not in sys.path and os.path.isdir(_p):
        sys.path.append(_p)

E, B, T, R, D = 8, 16, 512, 512, 8
NCORES = 8
BLK = 32          # timesteps per psum block
NBLK = T // BLK   # 16

# mm modes: 'sumap'  - one matmul per weight chunk, rhs=[sigma|tau], out AP
#                      broadcast so both halves accumulate into same 16 cols
#           '2mm'    - two matmuls per chunk (relies on walrus LDW dedupe)
#           'g'      - single g state, blend on critical path
MODE = os.environ.get("ESN_MODE", "sumap")
# reservoir-weight dtype: 'f16' | 'f8' (fp8-e4m3; no measured LDW win on HW)
WDT = os.environ.get("ESN_WDT", "f16")
# input-projection emission: 'burst' emits all 8 per-block xin matmuls at
# the block boundary; 'spread' interleaves them into the step stream
# (measured slower: mid-step win-LDWs break the wt-LDW pipeline)
XIN = os.environ.get("ESN_XIN", "burst")
# mm order: 'a' = qcA-consumers early (rc0/1 done by pos 9);
# 'b' = rc0/1 complete by pos 7 (earlier tanh0, tighter tanh1 margin);
# 'c' = 'b' order with one tanh per rc chunk right as its region completes
ORD = os.environ.get("ESN_ORD", "b")
# psum bank layout: 'rc' = rc-parity major (tanh src strided 3D);
# 'ctg' = step-major (tanh src one contiguous [128,32] slice)
LAYOUT = os.environ.get("ESN_LAYOUT", "rc")


def _build_nc(mode=MODE, timesteps=T, reps=1, wdt=WDT, xin=XIN, order=ORD,
              layout=LAYOUT, probe=None):
    # probe: timing-only ablations (output is wrong by design):
    #   'no_act'  - skip tanh activations
    #   'no_dve'  - skip sigma vector ops
    #   'half_w'  - only rc0/rc1 weight chunks (half the LDW stream)
    from contextlib import ExitStack

    import concourse.bass as bass  # noqa: F401
    import concourse.tile as tile
    from concourse import bacc, mybir

    f16 = mybir.dt.float16
    f32 = mybir.dt.float32
    fw = mybir.dt.float8e4 if wdt == "f8" else f16
    AF = mybir.ActivationFunctionType
    OP = mybir.AluOpType

    nc = bacc.Bacc(
        "TRN2",
        target_bir_lowering=False,
        debug=False,
        enable_asserts=False,
        num_devices=NCORES,
    )
    wt_d = nc.dram_tensor("wt", [128, 2048], fw, kind="ExternalInput").ap()
    win_d = nc.dram_tensor("win", [8, 512], f16, kind="ExternalInput").ap()
    xt_d = nc.dram_tensor("xt", [8, T * 16], f16, kind="ExternalInput").ap()
    ca_d = nc.dram_tensor("ca", [128, 2], f32, kind="ExternalInput").ap()
    out_d = nc.dram_tensor("out", [128, 64], f32, kind="ExternalOutput").ap()

    nblk = timesteps // BLK
    assert timesteps % BLK == 0

    with tile.TileContext(nc) as tc, ExitStack() as ctx:
        const = ctx.enter_context(tc.tile_pool(name="const", bufs=1))
        wt = const.tile([128, 2048], fw, tag="wt")
        win = const.tile([8, 512], f16, tag="win")
        xt = const.tile([8, T * 16], f16, tag="xt")
        ca = const.tile([128, 2], f32, tag="ca")
        nc.gpsimd.dma_start(wt[:], wt_d[:])
        nc.gpsimd.dma_start(win[:], win_d[:])
        nc.gpsimd.dma_start(xt[:], xt_d[:])
        nc.gpsimd.dma_start(ca[:], ca_d[:])

        statep = ctx.enter_context(tc.tile_pool(name="state", bufs=1))
        tmpp = ctx.enter_context(tc.tile_pool(name="tmp", bufs=2))
        psp = ctx.enter_context(tc.tile_pool(name="ps", bufs=1, space="PSUM"))
        ps = [psp.tile([128, 512], f32, name=f"ps{i}", tag=f"ps{i}") for i in range(8)]

        c_ap = ca[:, 0:1]
        a_ap = ca[:, 1:2]

        if mode in ("sumap", "2mm"):
            st = [statep.tile([128, 128], f16, name=f"st{i}", tag=f"st{i}") for i in range(2)]
        else:  # 'g'
            gt = [statep.tile([128, 64], f16, name=f"gt{i}", tag=f"g{i}") for i in range(2)]
            tt = [statep.tile([128, 64], f16, name=f"tt{i}", tag=f"t{i}") for i in range(2)]

        def bank(blk_i, half, par):
            return ps[(blk_i % 2) * 4 + half * 2 + par]

        def xin_mm_one(k, rcp, half, par):
            rc = half * 2 + rcp
            if layout == "ctg":
                out_ap = bank(k, half, par)[:].rearrange(
                    "p (i s) -> p i s", i=16)[:, :, rcp * 16:(rcp + 1) * 16]
            else:
                out_ap = bank(k, half, par)[:, rcp * 256:(rcp + 1) * 256]
            nc.tensor.matmul(
                out_ap,
                win[:, rc * 128:(rc + 1) * 128],
                xt[:, k * 512 + par * 256: k * 512 + (par + 1) * 256],
                start=(rcp == 0),
                stop=False,
                skip_group_check=True,
            )

        def xin_mms(k):
            # project x into psum banks for block k: u in fp32 psum
            for rcp in range(2):          # lhsT chunk; rc-major for LDW reuse
                for half in range(2):
                    for par in range(2):
                        xin_mm_one(k, rcp, half, par)

        # spread slots: (step-in-block, [(rcp, half, par), ...]) pairs
        # keeping par0/par1 adjacent for win-LDW reuse and all start=True
        # (rcp=0) writes in earlier slots than their rcp=1 accumulates
        XIN_SLOTS = {3 + 7 * j: [(j // 2, j % 2, 0), (j // 2, j % 2, 1)]
                     for j in range(4)}

        # feasible order: qcA-consumers early, qcB-consumers late, A-half
        # (rc0,rc1) groups complete by position 9
        if order == "a":
            MM_ORDER = [(0, 0), (0, 1), (1, 0), (1, 1), (2, 0), (3, 0),
                        (0, 2), (0, 3), (1, 2), (1, 3), (2, 1), (3, 1),
                        (2, 2), (2, 3), (3, 2), (3, 3)]
            RC_LAST = {0: 7, 1: 9, 2: 13, 3: 15}
        else:  # 'b'/'c': rc0/rc1 complete by pos 7
            MM_ORDER = [(0, 0), (0, 1), (1, 0), (1, 1), (0, 2), (0, 3),
                        (1, 2), (1, 3), (2, 0), (3, 0), (2, 1), (3, 1),
                        (2, 2), (2, 3), (3, 2), (3, 3)]
            RC_LAST = {0: 5, 1: 7, 2: 13, 3: 15}

        def body():
            if mode in ("sumap", "2mm"):
                nc.vector.memset(st[0][:], 0.0)
            else:
                nc.vector.memset(gt[0][:], 0.0)
            xin_mms(0)
            xin_mms(1)
            steps()
            finish()

        def steps():
         for t in range(timesteps):
            blk_i = t // BLK
            par = t % 2
            idx = (t % BLK) // 2
            if xin == "burst":
                if t % BLK == 0 and 1 <= blk_i and blk_i + 1 < nblk:
                    xin_mms(blk_i + 1)
            else:  # 'spread'
                if t % BLK in XIN_SLOTS and 1 <= blk_i and blk_i + 1 < nblk:
                    for rcp, half_x, par_x in XIN_SLOTS[t % BLK]:
                        xin_mm_one(blk_i + 1, rcp, half_x, par_x)

            if mode in ("sumap", "2mm"):
                so, sn = st[t % 2], st[(t + 1) % 2]
                so4 = so[:].rearrange("p (q s) -> p q s", q=4)
                sn4 = sn[:].rearrange("p (q s) -> p q s", q=4)
                # sigma' = c*(sigma+tau), off critical path
                if probe != "no_dve":
                    tmp = tmpp.tile([128, 64], f16, tag="tmp")
                    tmp3 = tmp[:].rearrange("p (q b) -> p q b", q=4)
                    nc.vector.tensor_add(tmp3, so4[:, :, 0:16], so4[:, :, 16:32])
                    nc.vector.tensor_scalar_mul(sn4[:, :, 0:16], tmp3, c_ap)

                def emit_mm(rc, qc):
                    half = rc // 2
                    if layout == "ctg":
                        colb = idx * 32 + (rc % 2) * 16
                    else:
                        colb = (rc % 2) * 256 + idx * 16
                    lhsT = wt[:, qc * 512 + rc * 128: qc * 512 + (rc + 1) * 128]
                    stop = RC_LAST[rc] == pos
                    outr = bank(blk_i, half, par)[:, colb:colb + 16]
                    if mode == "sumap":
                        out_ap = outr.unsqueeze(1).broadcast_to((128, 2, 16))
                        nc.tensor.matmul(
                            out_ap, lhsT, so[:, qc * 32:(qc + 1) * 32],
                            start=False, stop=stop, skip_group_check=True)
                    else:
                        nc.tensor.matmul(
                            outr, lhsT, so[:, qc * 32: qc * 32 + 16],
                            start=False, stop=False, skip_group_check=True)
                        nc.tensor.matmul(
                            outr, lhsT, so[:, qc * 32 + 16:(qc + 1) * 32],
                            start=False, stop=stop, skip_group_check=True)

                def emit_tanh(half):
                    b = bank(blk_i, half, par)
                    if layout == "ctg":
                        src = b[:].rearrange(
                            "p (i r b) -> p i r b", i=16, r=2)[:, idx, :, :]
                    else:
                        src = b[:].rearrange(
                            "p (r i b) -> p r i b", r=2, i=16)[:, :, idx, :]
                    dst = sn4[:, 2 * half: 2 * half + 2, 16:32]
                    nc.scalar.activation(dst, src, AF.Tanh)

                def emit_tanh_q(rc):
                    b = bank(blk_i, rc // 2, par)
                    if layout == "ctg":
                        src = b[:].rearrange(
                            "p (i r b) -> p i r b", i=16, r=2)[
                            :, idx, rc % 2: rc % 2 + 1, :]
                    else:
                        src = b[:].rearrange(
                            "p (r i b) -> p r i b", r=2, i=16)[
                            :, rc % 2: rc % 2 + 1, idx, :]
                    dst = sn4[:, rc: rc + 1, 16:32]
                    nc.scalar.activation(dst, src, AF.Tanh)

                if order == "c":
                    pos_tanh = {RC_LAST[rc]: rc for rc in range(4)}
                    for pos, (rc, qc) in enumerate(MM_ORDER):
                        emit_mm(rc, qc)
                        if pos in pos_tanh and pos != 15:
                            emit_tanh_q(pos_tanh[pos])
                    emit_tanh_q(3)
                else:
                    for pos, (rc, qc) in enumerate(MM_ORDER):
                        if probe == "half_w" and rc >= 2:
                            continue
                        emit_mm(rc, qc)
                        if pos == RC_LAST[1] and probe != "no_act":
                            emit_tanh(0)
                    if probe not in ("no_act", "half_w"):
                        emit_tanh(1)
            else:  # 'g' mode
                go, gn = gt[t % 2], gt[(t + 1) % 2]
                tn = tt[(t + 1) % 2]

                for pos, (rc, qc) in enumerate(MM_ORDER):
                    half = rc // 2
                    colb = (rc % 2) * 256 + idx * 16
                    nc.tensor.matmul(
                        bank(blk_i, half, par)[:, colb:colb + 16],
                        wt[:, qc * 512 + rc * 128: qc * 512 + (rc + 1) * 128],
                        go[:, qc * 16:(qc + 1) * 16],
                        start=False, stop=(RC_LAST[rc] == pos),
                        skip_group_check=True)
                    if pos == RC_LAST[1] or pos == RC_LAST[3]:
                        half = 0 if pos == RC_LAST[1] else 1
                        b = bank(blk_i, half, par)
                        src = b[:].rearrange("p (r i b) -> p r i b", r=2, i=16)[:, :, idx, :]
                        cols = slice(half * 32, half * 32 + 32)
                        nc.scalar.activation(tn[:, cols], src, AF.Tanh)
                        # g' = c*g + tau   (fused, on chain)
                        nc.vector.scalar_tensor_tensor(
                            gn[:, cols], go[:, cols], c_ap, tn[:, cols],
                            OP.mult, OP.add)

        def finish():
            # final: h = a * (sigma + tau)   [T even -> state in buffer 0]
            fin = timesteps % 2
            g32 = tmpp.tile([128, 64], f32, tag="g32")
            if mode in ("sumap", "2mm"):
                sf = st[fin][:].rearrange("p (q s) -> p q s", q=4)
                g3 = g32[:].rearrange("p (q b) -> p q b", q=4)
                nc.vector.tensor_add(g3, sf[:, :, 0:16], sf[:, :, 16:32])
            else:
                nc.vector.tensor_copy(g32[:], gt[fin][:])
            osb = tmpp.tile([128, 64], f32, tag="osb")
            nc.vector.tensor_scalar_mul(osb[:], g32[:], a_ap)
            nc.gpsimd.dma_start(out_d[:], osb[:])

        if reps == 1:
            body()
        else:
            # large body (>256 instructions/engine): hint the back-edge so
            # the branch target prefetches instead of a ~4us IRAM miss
            ET = mybir.EngineType
            with tc.For_i(0, reps, 1,
                          hint_engines=(ET.PE, ET.Activation, ET.DVE)):
                body()

    nc.compile()
    return nc


def _host_prep(x, W_in, W_res, lr, wdt=WDT):
    """Build the 8 per-core input maps."""
    import ml_dtypes

    wnp = ml_dtypes.float8_e4m3 if wdt == "f8" else np.float16
    x = np.asarray(x, np.float32)
    W_in = np.asarray(W_in, np.float32)
    W_res = np.asarray(W_res, np.float32)
    lr = np.asarray(lr, np.float32)

    # xt[d, blk*512 + par*256 + i*16 + b] = x[b, blk*32 + 2*i + par, d]
    xr = x.transpose(2, 1, 0)                     # [D, T, B]
    xr = xr.reshape(D, NBLK, BLK // 2, 2, B)      # [d, blk, i, par, b]
    xt = xr.transpose(0, 1, 3, 2, 4).reshape(D, T * 16)
    xt = np.ascontiguousarray(xt, np.float32).astype(np.float16)

    in_maps = []
    for e in range(NCORES):
        a = np.float32(lr[e])
        wtp = (a * W_res[e]).T                    # [q, r]
        wt = np.ascontiguousarray(
            wtp.reshape(4, 128, 512).transpose(1, 0, 2).reshape(128, 2048)
        ).astype(wnp)
        win = np.ascontiguousarray(W_in[e].T).astype(np.float16)  # [8, 512]
        ca = np.empty((128, 2), np.float32)
        ca[:, 0] = 1.0 - a
        ca[:, 1] = a
        in_maps.append({"wt": wt, "win": win, "xt": xt, "ca": ca})
    return in_maps


def _unshard(results):
    out = np.empty((B, E * R), np.float32)
    for e in range(NCORES):
        o = results[e]["out"]                      # [128, 64]
        he = o.reshape(128, 4, 16).transpose(2, 1, 0).reshape(B, R)
        out[:, e * R:(e + 1) * R] = he
    return out


def _run(in_maps, mode=MODE, trace=False, tmpdir=None):
    from concourse import bass_utils

    nc = _build_nc(mode=mode)
    res = bass_utils.run_bass_kernel_spmd(
        nc,
        in_maps,
        core_ids=list(range(NCORES)),
        trace=trace,
        tmpdir=tmpdir,
    )
    return res


_RUNNER = None


def _get_runner():
    """Compile once per process; repeat kernel() calls only re-upload inputs.

    Same lowering as bass2jax.run_bass_via_pjrt's multi-core path, but the
    jitted callable is kept so later calls skip the ~7s rebuild/recompile.
    No output donation: the kernel writes every element of 'out'.
    """
    global _RUNNER
    if _RUNNER is not None:
        return _RUNNER

    import jax
    from jax.sharding import Mesh, NamedSharding, PartitionSpec
    from jax.experimental.shard_map import shard_map
    from concourse import mybir
    from concourse.bass2jax import (
        _bass_exec_p, install_neuronx_cc_hook, partition_id_tensor)

    install_neuronx_cc_hook()
    nc = _build_nc(reps=1)
    partition_name = (
        nc.partition_id_tensor.name if nc.partition_id_tensor else None)
    in_names, out_names, out_avals, zero_outs = [], [], [], []
    for alloc in nc.m.functions[0].allocations:
        if not isinstance(alloc, mybir.MemoryLocationSet):
            continue
        name = alloc.memorylocations[0].name
        if alloc.kind == "ExternalInput":
            if name != partition_name:
                in_names.append(name)
        elif alloc.kind == "ExternalOutput":
            out_avals.append(jax.core.ShapedArray(
                tuple(alloc.tensor_shape), mybir.dt.np(alloc.dtype)))
            out_names.append(name)
            zero_outs.append(np.zeros(
                tuple(alloc.tensor_shape), mybir.dt.np(alloc.dtype)))
    n_params = len(in_names)
    all_in = list(in_names) + list(out_names) + (
        [partition_name] if partition_name else [])

    def _body(*args):
        operands = list(args)
        if partition_name:
            operands.append(partition_id_tensor())
        return tuple(_bass_exec_p.bind(
            *operands, out_avals=tuple(out_avals),
            in_names=tuple(all_in), out_names=tuple(out_names),
            lowering_input_output_aliases=(), sim_require_finite=True,
            sim_require_nnan=True, nc=nc))

    devices = jax.devices()[:NCORES]
    mesh = Mesh(np.asarray(devices), ("core",))
    fn = jax.jit(
        shard_map(
            _body, mesh=mesh,
            in_specs=(PartitionSpec("core"),) * (n_params + len(out_names)),
            out_specs=(PartitionSpec("core"),) * len(out_names),
            check_rep=False),
        keep_unused=True)
    sharding = NamedSharding(mesh, PartitionSpec("core"))
    dev_zeros = [
        jax.device_put(
            np.zeros((NCORES * z.shape[0], *z.shape[1:]), z.dtype), sharding)
        for z in zero_outs
    ]

    def run(in_maps):
        per_core = [[np.asarray(m[n]) for n in in_names] for m in in_maps]
        dev_in = [
            jax.device_put(
                np.concatenate(
                    [per_core[c][i] for c in range(NCORES)], axis=0),
                sharding)
            for i in range(n_params)
        ]
        outs = fn(*dev_in, *dev_zeros)
        return [
            {
                name: np.asarray(outs[i]).reshape(
                    NCORES, *out_avals[i].shape)[c]
                for i, name in enumerate(out_names)
            }
            for c in range(NCORES)
        ]

    _RUNNER = run
    return run


def kernel(x, W_in, W_res, lr):
    in_maps = _host_prep(x, W_in, W_res, lr)
    try:
        results = _get_runner()(in_maps)
    except Exception:
        global _RUNNER
        _RUNNER = None
        results = _run(in_maps, trace=False).results
    return _unshard(results)


if __name__ == "__main__":
    rng = np.random.default_rng(0)
    x = rng.normal(size=(B, T, D)).astype(np.float32)
    W_in = rng.normal(size=(E, R, D)).astype(np.float32) * 0.5
    W_res = (rng.normal(size=(E, R, R)) * (rng.random((E, R, R)) < 0.1)).astype(np.float32) * 0.05
    lr = rng.uniform(0.1, 0.5, E).astype(np.float32)
    out = kernel(x, W_in, W_res, lr)
    print("out", out.shape, out.dtype, np.abs(out).max())



# revision 35
# speedup vs baseline: 1.0026x; 1.0026x over previous
"""GroupedESN Trainium2 kernel.

Problem: E=8 echo-state networks, batch B=16, T=512 steps, reservoir R=512,
input D=8.  h_{t+1} = (1-a) h_t + a tanh(W_in x_t + W_res h_t), output is the
final state concatenated over ESNs -> [B, E*R].

Sharding: one ESN per NeuronCore (8 cores).  Inside a core the recurrence is
sequential over T; per step the tensor engine re-ingests W (fp16 stationary,
fast-weight-load) as 16 [128,128] chunks.

State substitution (a folded into W, so per-core program is data-independent):
  g = h / a,  W'' = a * W_res,  c = 1 - a
  g_{t+1} = c g_t + tanh(u_t + W'' g_t)
Split g = sigma + tau so the only serial op between steps is the tanh:
  tau_{t+1}   = tanh(u_t + W'' sigma_t + W'' tau_t)     (scalar engine)
  sigma_{t+1} = c (sigma_t + tau_t)                     (vector, off-chain)

PSUM layout: 8 banks = (block parity) x (rc half) x (step parity).  Input
projections u_t are matmul'd directly into the banks (start=True), recurrence
matmuls accumulate on top (start=False), tanh reads PSUM.

Measured on HW (reps-loop slope, see test.py): ~513us/exec = ~1.0us/step.
Ablations: PE stream alone (no tanh) is 656ns/step; the remaining ~340ns is
the tanh chain (ACT ~260ns fixed cost + 2 sem crossings + the 8 chain-pair
LDW stream, which cannot prefetch past a sem-blocked matmul - one background
weight buffer).  fp8 weights, xin spreading, mm reorders, quarter-tanhs
(ACT-saturates), and contiguous psum layouts were all measured neutral or
worse; flags below keep those variants reproducible.
"""

import os
import sys

import numpy as np

for _p in ("/opt/trn_rl_repo", "/root/.axon_site/_ro/trn_rl_repo"):
    if _p not in sys.path and os.path.isdir(_p):
        sys.path.append(_p)

E, B, T, R, D = 8, 16, 512, 512, 8
NCORES = 8
BLK = 32          # timesteps per psum block
NBLK = T // BLK   # 16

# mm modes: 'sumap'  - one matmul per weight chunk, rhs=[sigma|tau], out AP
#                      broadcast so both halves accumulate into same 16 cols
#           '2mm'    - two matmuls per chunk (relies on walrus LDW dedupe)
#           'g'      - single g state, blend on critical path
MODE = os.environ.get("ESN_MODE", "sumap")
# reservoir-weight dtype: 'f16' | 'f8' (fp8-e4m3; no measured LDW win on HW)
WDT = os.environ.get("ESN_WDT", "f16")
# input-projection emission: 'burst' emits all 8 per-block xin matmuls at
# the block boundary; 'spread' interleaves them into the step stream
# (measured slower: mid-step win-LDWs break the wt-LDW pipeline)
XIN = os.environ.get("ESN_XIN", "burst")
# mm order: 'a' = qcA-consumers early (rc0/1 done by pos 9);
# 'b' = rc0/1 complete by pos 7 (earlier tanh0, tighter tanh1 margin);
# 'c' = 'b' order with one tanh per rc chunk right as its region completes
ORD = os.environ.get("ESN_ORD", "b")
# psum bank layout: 'rc' = rc-parity major (tanh src strided 3D);
# 'ctg' = step-major (tanh src one contiguous [128,32] slice)
LAYOUT = os.environ.get("ESN_LAYOUT", "rc")


def _build_nc(mode=MODE, timesteps=T, reps=1, wdt=WDT, xin=XIN, order=ORD,
              layout=LAYOUT, probe=None, stagger=False):
    # probe: timing-only ablations (output is wrong by design):
    #   'no_act'  - skip tanh activations
    #   'no_dve'  - skip sigma vector ops
    #   'half_w'  - only rc0/rc1 weight chunks (half the LDW stream)
    from contextlib import ExitStack

    import concourse.bass as bass  # noqa: F401
    import concourse.tile as tile
    from concourse import bacc, mybir

    f16 = mybir.dt.float16
    f32 = mybir.dt.float32
    fw = mybir.dt.float8e4 if wdt == "f8" else f16
    AF = mybir.ActivationFunctionType
    OP = mybir.AluOpType

    nc = bacc.Bacc(
        "TRN2",
        target_bir_lowering=False,
        debug=False,
        enable_asserts=False,
        num_devices=NCORES,
    )
    wt_d = nc.dram_tensor("wt", [128, 2048], fw, kind="ExternalInput").ap()
    win_d = nc.dram_tensor("win", [8, 512], f16, kind="ExternalInput").ap()
    xt_d = nc.dram_tensor("xt", [8, T * 16], f16, kind="ExternalInput").ap()
    ca_d = nc.dram_tensor("ca", [128, 2], f32, kind="ExternalInput").ap()
    out_d = nc.dram_tensor("out", [128, 64], f32, kind="ExternalOutput").ap()

    nblk = timesteps // BLK
    assert timesteps % BLK == 0

    with tile.TileContext(nc) as tc, ExitStack() as ctx:
        const = ctx.enter_context(tc.tile_pool(name="const", bufs=1))
        wt = const.tile([128, 2048], fw, tag="wt")
        win = const.tile([8, 512], f16, tag="win")
        xt = const.tile([8, T * 16], f16, tag="xt")
        ca = const.tile([128, 2], f32, tag="ca")
        nc.gpsimd.dma_start(wt[:], wt_d[:])
        nc.gpsimd.dma_start(win[:], win_d[:])
        nc.gpsimd.dma_start(xt[:], xt_d[:])
        nc.gpsimd.dma_start(ca[:], ca_d[:])

        statep = ctx.enter_context(tc.tile_pool(name="state", bufs=1))
        tmpp = ctx.enter_context(tc.tile_pool(name="tmp", bufs=2))
        psp = ctx.enter_context(tc.tile_pool(name="ps", bufs=1, space="PSUM"))
        ps = [psp.tile([128, 512], f32, name=f"ps{i}", tag=f"ps{i}") for i in range(8)]

        c_ap = ca[:, 0:1]
        a_ap = ca[:, 1:2]

        if mode in ("sumap", "2mm"):
            st = [statep.tile([128, 128], f16, name=f"st{i}", tag=f"st{i}") for i in range(2)]
        else:  # 'g'
            gt = [statep.tile([128, 64], f16, name=f"gt{i}", tag=f"g{i}") for i in range(2)]
            tt = [statep.tile([128, 64], f16, name=f"tt{i}", tag=f"t{i}") for i in range(2)]

        def bank(blk_i, half, par):
            return ps[(blk_i % 2) * 4 + half * 2 + par]

        def xin_mm_one(k, rcp, half, par):
            rc = half * 2 + rcp
            if layout == "ctg":
                out_ap = bank(k, half, par)[:].rearrange(
                    "p (i s) -> p i s", i=16)[:, :, rcp * 16:(rcp + 1) * 16]
            else:
                out_ap = bank(k, half, par)[:, rcp * 256:(rcp + 1) * 256]
            nc.tensor.matmul(
                out_ap,
                win[:, rc * 128:(rc + 1) * 128],
                xt[:, k * 512 + par * 256: k * 512 + (par + 1) * 256],
                start=(rcp == 0),
                stop=False,
                skip_group_check=True,
            )

        def xin_mms(k):
            # project x into psum banks for block k: u in fp32 psum
            for rcp in range(2):          # lhsT chunk; rc-major for LDW reuse
                for half in range(2):
                    for par in range(2):
                        xin_mm_one(k, rcp, half, par)

        # spread slots: (step-in-block, [(rcp, half, par), ...]) pairs
        # keeping par0/par1 adjacent for win-LDW reuse and all start=True
        # (rcp=0) writes in earlier slots than their rcp=1 accumulates
        XIN_SLOTS = {3 + 7 * j: [(j // 2, j % 2, 0), (j // 2, j % 2, 1)]
                     for j in range(4)}

        # feasible order: qcA-consumers early, qcB-consumers late, A-half
        # (rc0,rc1) groups complete by position 9
        if order == "a":
            MM_ORDER = [(0, 0), (0, 1), (1, 0), (1, 1), (2, 0), (3, 0),
                        (0, 2), (0, 3), (1, 2), (1, 3), (2, 1), (3, 1),
                        (2, 2), (2, 3), (3, 2), (3, 3)]
            RC_LAST = {0: 7, 1: 9, 2: 13, 3: 15}
        else:  # 'b'/'c': rc0/rc1 complete by pos 7
            MM_ORDER = [(0, 0), (0, 1), (1, 0), (1, 1), (0, 2), (0, 3),
                        (1, 2), (1, 3), (2, 0), (3, 0), (2, 1), (3, 1),
                        (2, 2), (2, 3), (3, 2), (3, 3)]
            RC_LAST = {0: 5, 1: 7, 2: 13, 3: 15}

        def body():
            if mode in ("sumap", "2mm"):
                nc.vector.memset(st[0][:], 0.0)
            else:
                nc.vector.memset(gt[0][:], 0.0)
            xin_mms(0)
            xin_mms(1)
            steps()
            finish()

        def steps():
         for t in range(timesteps):
            blk_i = t // BLK
            par = t % 2
            idx = (t % BLK) // 2
            if xin == "burst":
                if t % BLK == 0 and 1 <= blk_i and blk_i + 1 < nblk:
                    xin_mms(blk_i + 1)
            else:  # 'spread'
                if t % BLK in XIN_SLOTS and 1 <= blk_i and blk_i + 1 < nblk:
                    for rcp, half_x, par_x in XIN_SLOTS[t % BLK]:
                        xin_mm_one(blk_i + 1, rcp, half_x, par_x)

            if mode in ("sumap", "2mm"):
                so, sn = st[t % 2], st[(t + 1) % 2]
                so4 = so[:].rearrange("p (q s) -> p q s", q=4)
                sn4 = sn[:].rearrange("p (q s) -> p q s", q=4)
                # sigma' = c*(sigma+tau), off critical path
                if probe != "no_dve":
                    tmp = tmpp.tile([128, 64], f16, tag="tmp")
                    tmp3 = tmp[:].rearrange("p (q b) -> p q b", q=4)
                    nc.vector.tensor_add(tmp3, so4[:, :, 0:16], so4[:, :, 16:32])
                    nc.vector.tensor_scalar_mul(sn4[:, :, 0:16], tmp3, c_ap)

                def emit_mm(rc, qc):
                    half = rc // 2
                    if layout == "ctg":
                        colb = idx * 32 + (rc % 2) * 16
                    else:
                        colb = (rc % 2) * 256 + idx * 16
                    lhsT = wt[:, qc * 512 + rc * 128: qc * 512 + (rc + 1) * 128]
                    stop = RC_LAST[rc] == pos
                    outr = bank(blk_i, half, par)[:, colb:colb + 16]
                    if mode == "sumap":
                        out_ap = outr.unsqueeze(1).broadcast_to((128, 2, 16))
                        nc.tensor.matmul(
                            out_ap, lhsT, so[:, qc * 32:(qc + 1) * 32],
                            start=False, stop=stop, skip_group_check=True)
                    else:
                        nc.tensor.matmul(
                            outr, lhsT, so[:, qc * 32: qc * 32 + 16],
                            start=False, stop=False, skip_group_check=True)
                        nc.tensor.matmul(
                            outr, lhsT, so[:, qc * 32 + 16:(qc + 1) * 32],
                            start=False, stop=stop, skip_group_check=True)

                def emit_tanh(half):
                    b = bank(blk_i, half, par)
                    if layout == "ctg":
                        src = b[:].rearrange(
                            "p (i r b) -> p i r b", i=16, r=2)[:, idx, :, :]
                    else:
                        src = b[:].rearrange(
                            "p (r i b) -> p r i b", r=2, i=16)[:, :, idx, :]
                    dst = sn4[:, 2 * half: 2 * half + 2, 16:32]
                    nc.scalar.activation(dst, src, AF.Tanh)

                def emit_tanh_q(rc):
                    b = bank(blk_i, rc // 2, par)
                    if layout == "ctg":
                        src = b[:].rearrange(
                            "p (i r b) -> p i r b", i=16, r=2)[
                            :, idx, rc % 2: rc % 2 + 1, :]
                    else:
                        src = b[:].rearrange(
                            "p (r i b) -> p r i b", r=2, i=16)[
                            :, rc % 2: rc % 2 + 1, idx, :]
                    dst = sn4[:, rc: rc + 1, 16:32]
                    nc.scalar.activation(dst, src, AF.Tanh)

                if order == "c":
                    pos_tanh = {RC_LAST[rc]: rc for rc in range(4)}
                    for pos, (rc, qc) in enumerate(MM_ORDER):
                        emit_mm(rc, qc)
                        if pos in pos_tanh and pos != 15:
                            emit_tanh_q(pos_tanh[pos])
                    emit_tanh_q(3)
                else:
                    for pos, (rc, qc) in enumerate(MM_ORDER):
                        if probe == "half_w" and rc >= 2:
                            continue
                        emit_mm(rc, qc)
                        if pos == RC_LAST[1] and probe != "no_act":
                            emit_tanh(0)
                    if probe not in ("no_act", "half_w"):
                        emit_tanh(1)
            else:  # 'g' mode
                go, gn = gt[t % 2], gt[(t + 1) % 2]
                tn = tt[(t + 1) % 2]

                for pos, (rc, qc) in enumerate(MM_ORDER):
                    half = rc // 2
                    colb = (rc % 2) * 256 + idx * 16
                    nc.tensor.matmul(
                        bank(blk_i, half, par)[:, colb:colb + 16],
                        wt[:, qc * 512 + rc * 128: qc * 512 + (rc + 1) * 128],
                        go[:, qc * 16:(qc + 1) * 16],
                        start=False, stop=(RC_LAST[rc] == pos),
                        skip_group_check=True)
                    if pos == RC_LAST[1] or pos == RC_LAST[3]:
                        half = 0 if pos == RC_LAST[1] else 1
                        b = bank(blk_i, half, par)
                        src = b[:].rearrange("p (r i b) -> p r i b", r=2, i=16)[:, :, idx, :]
                        cols = slice(half * 32, half * 32 + 32)
                        nc.scalar.activation(tn[:, cols], src, AF.Tanh)
                        # g' = c*g + tau   (fused, on chain)
                        nc.vector.scalar_tensor_tensor(
                            gn[:, cols], go[:, cols], c_ap, tn[:, cols],
                            OP.mult, OP.add)

        def finish():
            # final: h = a * (sigma + tau)   [T even -> state in buffer 0]
            fin = timesteps % 2
            g32 = tmpp.tile([128, 64], f32, tag="g32")
            if mode in ("sumap", "2mm"):
                sf = st[fin][:].rearrange("p (q s) -> p q s", q=4)
                g3 = g32[:].rearrange("p (q b) -> p q b", q=4)
                nc.vector.tensor_add(g3, sf[:, :, 0:16], sf[:, :, 16:32])
            else:
                nc.vector.tensor_copy(g32[:], gt[fin][:])
            osb = tmpp.tile([128, 64], f32, tag="osb")
            nc.vector.tensor_scalar_mul(osb[:], g32[:], a_ap)
            nc.gpsimd.dma_start(out_d[:], osb[:])

        if reps == 1:
            body()
        else:
            # large body (>256 instructions/engine): hint the back-edge so
            # the branch target prefetches instead of a ~4us IRAM miss
            ET = mybir.EngineType
            with tc.For_i(0, reps, 1,
                          hint_engines=(ET.PE, ET.Activation, ET.DVE),
                          staggered_reset=stagger):
                body()

    nc.compile()
    return nc


def _host_prep(x, W_in, W_res, lr, wdt=WDT):
    """Build the 8 per-core input maps."""
    import ml_dtypes

    wnp = ml_dtypes.float8_e4m3 if wdt == "f8" else np.float16
    x = np.asarray(x, np.float32)
    W_in = np.asarray(W_in, np.float32)
    W_res = np.asarray(W_res, np.float32)
    lr = np.asarray(lr, np.float32)

    # xt[d, blk*512 + par*256 + i*16 + b] = x[b, blk*32 + 2*i + par, d]
    xr = x.transpose(2, 1, 0)                     # [D, T, B]
    xr = xr.reshape(D, NBLK, BLK // 2, 2, B)      # [d, blk, i, par, b]
    xt = xr.transpose(0, 1, 3, 2, 4).reshape(D, T * 16)
    xt = np.ascontiguousarray(xt, np.float32).astype(np.float16)

    in_maps = []
    for e in range(NCORES):
        a = np.float32(lr[e])
        wtp = (a * W_res[e]).T                    # [q, r]
        wt = np.ascontiguousarray(
            wtp.reshape(4, 128, 512).transpose(1, 0, 2).reshape(128, 2048)
        ).astype(wnp)
        win = np.ascontiguousarray(W_in[e].T).astype(np.float16)  # [8, 512]
        ca = np.empty((128, 2), np.float32)
        ca[:, 0] = 1.0 - a
        ca[:, 1] = a
        in_maps.append({"wt": wt, "win": win, "xt": xt, "ca": ca})
    return in_maps


def _unshard(results):
    out = np.empty((B, E * R), np.float32)
    for e in range(NCORES):
        o = results[e]["out"]                      # [128, 64]
        he = o.reshape(128, 4, 16).transpose(2, 1, 0).reshape(B, R)
        out[:, e * R:(e + 1) * R] = he
    return out


def _run(in_maps, mode=MODE, trace=False, tmpdir=None):
    from concourse import bass_utils

    nc = _build_nc(mode=mode)
    res = bass_utils.run_bass_kernel_spmd(
        nc,
        in_maps,
        core_ids=list(range(NCORES)),
        trace=trace,
        tmpdir=tmpdir,
    )
    return res


_RUNNER = None


def _get_runner():
    """Compile once per process; repeat kernel() calls only re-upload inputs.

    Same lowering as bass2jax.run_bass_via_pjrt's multi-core path, but the
    jitted callable is kept so later calls skip the ~7s rebuild/recompile.
    No output donation: the kernel writes every element of 'out'.
    """
    global _RUNNER
    if _RUNNER is not None:
        return _RUNNER

    import jax
    from jax.sharding import Mesh, NamedSharding, PartitionSpec
    from jax.experimental.shard_map import shard_map
    from concourse import mybir
    from concourse.bass2jax import (
        _bass_exec_p, install_neuronx_cc_hook, partition_id_tensor)

    install_neuronx_cc_hook()
    nc = _build_nc(reps=1)
    partition_name = (
        nc.partition_id_tensor.name if nc.partition_id_tensor else None)
    in_names, out_names, out_avals, zero_outs = [], [], [], []
    for alloc in nc.m.functions[0].allocations:
        if not isinstance(alloc, mybir.MemoryLocationSet):
            continue
        name = alloc.memorylocations[0].name
        if alloc.kind == "ExternalInput":
            if name != partition_name:
                in_names.append(name)
        elif alloc.kind == "ExternalOutput":
            out_avals.append(jax.core.ShapedArray(
                tuple(alloc.tensor_shape), mybir.dt.np(alloc.dtype)))
            out_names.append(name)
            zero_outs.append(np.zeros(
                tuple(alloc.tensor_shape), mybir.dt.np(alloc.dtype)))
    n_params = len(in_names)
    all_in = list(in_names) + list(out_names) + (
        [partition_name] if partition_name else [])

    def _body(*args):
        operands = list(args)
        if partition_name:
            operands.append(partition_id_tensor())
        return tuple(_bass_exec_p.bind(
            *operands, out_avals=tuple(out_avals),
            in_names=tuple(all_in), out_names=tuple(out_names),
            lowering_input_output_aliases=(), sim_require_finite=True,
            sim_require_nnan=True, nc=nc))

    devices = jax.devices()[:NCORES]
    mesh = Mesh(np.asarray(devices), ("core",))
    fn = jax.jit(
        shard_map(
            _body, mesh=mesh,
            in_specs=(PartitionSpec("core"),) * (n_params + len(out_names)),
            out_specs=(PartitionSpec("core"),) * len(out_names),
            check_rep=False),
        keep_unused=True)
    sharding = NamedSharding(mesh, PartitionSpec("core"))
    dev_zeros = [
        jax.device_put(
            np.zeros((NCORES * z.shape[0], *z.shape[1:]), z.dtype), sharding)
        for z in zero_outs
    ]

    def run(in_maps):
        per_core = [[np.asarray(m[n]) for n in in_names] for m in in_maps]
        dev_in = [
            jax.device_put(
                np.concatenate(
                    [per_core[c][i] for c in range(NCORES)], axis=0),
                sharding)
            for i in range(n_params)
        ]
        outs = fn(*dev_in, *dev_zeros)
        return [
            {
                name: np.asarray(outs[i]).reshape(
                    NCORES, *out_avals[i].shape)[c]
                for i, name in enumerate(out_names)
            }
            for c in range(NCORES)
        ]

    _RUNNER = run
    return run


def kernel(x, W_in, W_res, lr):
    in_maps = _host_prep(x, W_in, W_res, lr)
    try:
        results = _get_runner()(in_maps)
    except Exception:
        global _RUNNER
        _RUNNER = None
        results = _run(in_maps, trace=False).results
    return _unshard(results)


if __name__ == "__main__":
    rng = np.random.default_rng(0)
    x = rng.normal(size=(B, T, D)).astype(np.float32)
    W_in = rng.normal(size=(E, R, D)).astype(np.float32) * 0.5
    W_res = (rng.normal(size=(E, R, R)) * (rng.random((E, R, R)) < 0.1)).astype(np.float32) * 0.05
    lr = rng.uniform(0.1, 0.5, E).astype(np.float32)
    out = kernel(x, W_in, W_res, lr)
    print("out", out.shape, out.dtype, np.abs(out).max())



# revision 36
# speedup vs baseline: 1.0038x; 1.0012x over previous
"""GroupedESN Trainium2 kernel.

Problem: E=8 echo-state networks, batch B=16, T=512 steps, reservoir R=512,
input D=8.  h_{t+1} = (1-a) h_t + a tanh(W_in x_t + W_res h_t), output is the
final state concatenated over ESNs -> [B, E*R].

Sharding: one ESN per NeuronCore (8 cores).  Inside a core the recurrence is
sequential over T; per step the tensor engine re-ingests W (fp16 stationary,
fast-weight-load) as 16 [128,128] chunks.

State substitution (a folded into W, so per-core program is data-independent):
  g = h / a,  W'' = a * W_res,  c = 1 - a
  g_{t+1} = c g_t + tanh(u_t + W'' g_t)
Split g = sigma + tau so the only serial op between steps is the tanh:
  tau_{t+1}   = tanh(u_t + W'' sigma_t + W'' tau_t)     (scalar engine)
  sigma_{t+1} = c (sigma_t + tau_t)                     (vector, off-chain)

PSUM layout: 8 banks = (block parity) x (rc half) x (step parity).  Input
projections u_t are matmul'd directly into the banks (start=True), recurrence
matmuls accumulate on top (start=False), tanh reads PSUM.

Measured on HW (reps-loop slope, see test.py): ~512us/exec = ~1.0us/step.
Ablations: PE stream alone (no tanh) is 656ns/step; the remaining ~340ns is
the tanh chain.  Validated loop model (993 measured vs 954 modeled):
  sem(100) + 4 qc01-pairs(164) + tanh1-queue stall(~96) + 4 qc23-pairs(164)
  + psum drain(170) + ACT tanh(260)
The 8 chain LDWs cannot preload past a sem-blocked matmul (one background
weight buffer, in-order NX dispatch; tile_position splits die on the psum
partition <-> array col-group hard wiring), and any MM reordering sits on
the same max(8 pairs, ACT-serialization + 4 pairs) envelope - order 'b' is
at the optimum.  Measured neutral or worse: fp8 weights (FWL weight-load
bandwidth is dtype-independent: no-tanh probe 655.7 vs 657.2 ns/step), xin
spreading, mm reorders, quarter-tanhs (ACT fixed cost ~260-350ns saturates
at 4/step), contiguous psum layouts, staggered_reset back-edges.  Flags
below keep those variants reproducible.
"""

import os
import sys

import numpy as np

for _p in ("/opt/trn_rl_repo", "/root/.axon_site/_ro/trn_rl_repo"):
    if _p not in sys.path and os.path.isdir(_p):
        sys.path.append(_p)

E, B, T, R, D = 8, 16, 512, 512, 8
NCORES = 8
BLK = 32          # timesteps per psum block
NBLK = T // BLK   # 16

# mm modes: 'sumap'  - one matmul per weight chunk, rhs=[sigma|tau], out AP
#                      broadcast so both halves accumulate into same 16 cols
#           '2mm'    - two matmuls per chunk (relies on walrus LDW dedupe)
#           'g'      - single g state, blend on critical path
MODE = os.environ.get("ESN_MODE", "sumap")
# reservoir-weight dtype: 'f16' | 'f8' (fp8-e4m3; no measured LDW win on HW)
WDT = os.environ.get("ESN_WDT", "f16")
# input-projection emission: 'burst' emits all 8 per-block xin matmuls at
# the block boundary; 'spread' interleaves them into the step stream
# (measured slower: mid-step win-LDWs break the wt-LDW pipeline)
XIN = os.environ.get("ESN_XIN", "burst")
# mm order: 'a' = qcA-consumers early (rc0/1 done by pos 9);
# 'b' = rc0/1 complete by pos 7 (earlier tanh0, tighter tanh1 margin);
# 'c' = 'b' order with one tanh per rc chunk right as its region completes
ORD = os.environ.get("ESN_ORD", "b")
# psum bank layout: 'rc' = rc-parity major (tanh src strided 3D);
# 'ctg' = step-major (tanh src one contiguous [128,32] slice)
LAYOUT = os.environ.get("ESN_LAYOUT", "rc")


def _build_nc(mode=MODE, timesteps=T, reps=1, wdt=WDT, xin=XIN, order=ORD,
              layout=LAYOUT, probe=None, stagger=False):
    # probe: timing-only ablations (output is wrong by design):
    #   'no_act'  - skip tanh activations
    #   'no_dve'  - skip sigma vector ops
    #   'half_w'  - only rc0/rc1 weight chunks (half the LDW stream)
    from contextlib import ExitStack

    import concourse.bass as bass  # noqa: F401
    import concourse.tile as tile
    from concourse import bacc, mybir

    f16 = mybir.dt.float16
    f32 = mybir.dt.float32
    fw = mybir.dt.float8e4 if wdt == "f8" else f16
    AF = mybir.ActivationFunctionType
    OP = mybir.AluOpType

    nc = bacc.Bacc(
        "TRN2",
        target_bir_lowering=False,
        debug=False,
        enable_asserts=False,
        num_devices=NCORES,
    )
    wt_d = nc.dram_tensor("wt", [128, 2048], fw, kind="ExternalInput").ap()
    win_d = nc.dram_tensor("win", [8, 512], f16, kind="ExternalInput").ap()
    xt_d = nc.dram_tensor("xt", [8, T * 16], f16, kind="ExternalInput").ap()
    ca_d = nc.dram_tensor("ca", [128, 2], f32, kind="ExternalInput").ap()
    out_d = nc.dram_tensor("out", [128, 64], f32, kind="ExternalOutput").ap()

    nblk = timesteps // BLK
    assert timesteps % BLK == 0

    with tile.TileContext(nc) as tc, ExitStack() as ctx:
        const = ctx.enter_context(tc.tile_pool(name="const", bufs=1))
        wt = const.tile([128, 2048], fw, tag="wt")
        win = const.tile([8, 512], f16, tag="win")
        xt = const.tile([8, T * 16], f16, tag="xt")
        ca = const.tile([128, 2], f32, tag="ca")
        nc.gpsimd.dma_start(wt[:], wt_d[:])
        nc.gpsimd.dma_start(win[:], win_d[:])
        nc.gpsimd.dma_start(xt[:], xt_d[:])
        nc.gpsimd.dma_start(ca[:], ca_d[:])

        statep = ctx.enter_context(tc.tile_pool(name="state", bufs=1))
        tmpp = ctx.enter_context(tc.tile_pool(name="tmp", bufs=2))
        psp = ctx.enter_context(tc.tile_pool(name="ps", bufs=1, space="PSUM"))
        ps = [psp.tile([128, 512], f32, name=f"ps{i}", tag=f"ps{i}") for i in range(8)]

        c_ap = ca[:, 0:1]
        a_ap = ca[:, 1:2]

        if mode in ("sumap", "2mm"):
            st = [statep.tile([128, 128], f16, name=f"st{i}", tag=f"st{i}") for i in range(2)]
        else:  # 'g'
            gt = [statep.tile([128, 64], f16, name=f"gt{i}", tag=f"g{i}") for i in range(2)]
            tt = [statep.tile([128, 64], f16, name=f"tt{i}", tag=f"t{i}") for i in range(2)]

        def bank(blk_i, half, par):
            return ps[(blk_i % 2) * 4 + half * 2 + par]

        def xin_mm_one(k, rcp, half, par):
            rc = half * 2 + rcp
            if layout == "ctg":
                out_ap = bank(k, half, par)[:].rearrange(
                    "p (i s) -> p i s", i=16)[:, :, rcp * 16:(rcp + 1) * 16]
            else:
                out_ap = bank(k, half, par)[:, rcp * 256:(rcp + 1) * 256]
            nc.tensor.matmul(
                out_ap,
                win[:, rc * 128:(rc + 1) * 128],
                xt[:, k * 512 + par * 256: k * 512 + (par + 1) * 256],
                start=(rcp == 0),
                stop=False,
                skip_group_check=True,
            )

        def xin_mms(k):
            # project x into psum banks for block k: u in fp32 psum
            for rcp in range(2):          # lhsT chunk; rc-major for LDW reuse
                for half in range(2):
                    for par in range(2):
                        xin_mm_one(k, rcp, half, par)

        # spread slots: (step-in-block, [(rcp, half, par), ...]) pairs
        # keeping par0/par1 adjacent for win-LDW reuse and all start=True
        # (rcp=0) writes in earlier slots than their rcp=1 accumulates
        XIN_SLOTS = {3 + 7 * j: [(j // 2, j % 2, 0), (j // 2, j % 2, 1)]
                     for j in range(4)}

        # feasible order: qcA-consumers early, qcB-consumers late, A-half
        # (rc0,rc1) groups complete by position 9
        if order == "a":
            MM_ORDER = [(0, 0), (0, 1), (1, 0), (1, 1), (2, 0), (3, 0),
                        (0, 2), (0, 3), (1, 2), (1, 3), (2, 1), (3, 1),
                        (2, 2), (2, 3), (3, 2), (3, 3)]
            RC_LAST = {0: 7, 1: 9, 2: 13, 3: 15}
        else:  # 'b'/'c': rc0/rc1 complete by pos 7
            MM_ORDER = [(0, 0), (0, 1), (1, 0), (1, 1), (0, 2), (0, 3),
                        (1, 2), (1, 3), (2, 0), (3, 0), (2, 1), (3, 1),
                        (2, 2), (2, 3), (3, 2), (3, 3)]
            RC_LAST = {0: 5, 1: 7, 2: 13, 3: 15}

        def body():
            if mode in ("sumap", "2mm"):
                nc.vector.memset(st[0][:], 0.0)
            else:
                nc.vector.memset(gt[0][:], 0.0)
            xin_mms(0)
            xin_mms(1)
            steps()
            finish()

        def steps():
         for t in range(timesteps):
            blk_i = t // BLK
            par = t % 2
            idx = (t % BLK) // 2
            if xin == "burst":
                if t % BLK == 0 and 1 <= blk_i and blk_i + 1 < nblk:
                    xin_mms(blk_i + 1)
            else:  # 'spread'
                if t % BLK in XIN_SLOTS and 1 <= blk_i and blk_i + 1 < nblk:
                    for rcp, half_x, par_x in XIN_SLOTS[t % BLK]:
                        xin_mm_one(blk_i + 1, rcp, half_x, par_x)

            if mode in ("sumap", "2mm"):
                so, sn = st[t % 2], st[(t + 1) % 2]
                so4 = so[:].rearrange("p (q s) -> p q s", q=4)
                sn4 = sn[:].rearrange("p (q s) -> p q s", q=4)
                # sigma' = c*(sigma+tau), off critical path
                if probe != "no_dve":
                    tmp = tmpp.tile([128, 64], f16, tag="tmp")
                    tmp3 = tmp[:].rearrange("p (q b) -> p q b", q=4)
                    nc.vector.tensor_add(tmp3, so4[:, :, 0:16], so4[:, :, 16:32])
                    nc.vector.tensor_scalar_mul(sn4[:, :, 0:16], tmp3, c_ap)

                def emit_mm(rc, qc):
                    half = rc // 2
                    if layout == "ctg":
                        colb = idx * 32 + (rc % 2) * 16
                    else:
                        colb = (rc % 2) * 256 + idx * 16
                    lhsT = wt[:, qc * 512 + rc * 128: qc * 512 + (rc + 1) * 128]
                    stop = RC_LAST[rc] == pos
                    outr = bank(blk_i, half, par)[:, colb:colb + 16]
                    if mode == "sumap":
                        out_ap = outr.unsqueeze(1).broadcast_to((128, 2, 16))
                        nc.tensor.matmul(
                            out_ap, lhsT, so[:, qc * 32:(qc + 1) * 32],
                            start=False, stop=stop, skip_group_check=True)
                    else:
                        nc.tensor.matmul(
                            outr, lhsT, so[:, qc * 32: qc * 32 + 16],
                            start=False, stop=False, skip_group_check=True)
                        nc.tensor.matmul(
                            outr, lhsT, so[:, qc * 32 + 16:(qc + 1) * 32],
                            start=False, stop=stop, skip_group_check=True)

                def emit_tanh(half):
                    b = bank(blk_i, half, par)
                    if layout == "ctg":
                        src = b[:].rearrange(
                            "p (i r b) -> p i r b", i=16, r=2)[:, idx, :, :]
                    else:
                        src = b[:].rearrange(
                            "p (r i b) -> p r i b", r=2, i=16)[:, :, idx, :]
                    dst = sn4[:, 2 * half: 2 * half + 2, 16:32]
                    nc.scalar.activation(dst, src, AF.Tanh)

                def emit_tanh_q(rc):
                    b = bank(blk_i, rc // 2, par)
                    if layout == "ctg":
                        src = b[:].rearrange(
                            "p (i r b) -> p i r b", i=16, r=2)[
                            :, idx, rc % 2: rc % 2 + 1, :]
                    else:
                        src = b[:].rearrange(
                            "p (r i b) -> p r i b", r=2, i=16)[
                            :, rc % 2: rc % 2 + 1, idx, :]
                    dst = sn4[:, rc: rc + 1, 16:32]
                    nc.scalar.activation(dst, src, AF.Tanh)

                if order == "c":
                    pos_tanh = {RC_LAST[rc]: rc for rc in range(4)}
                    for pos, (rc, qc) in enumerate(MM_ORDER):
                        emit_mm(rc, qc)
                        if pos in pos_tanh and pos != 15:
                            emit_tanh_q(pos_tanh[pos])
                    emit_tanh_q(3)
                else:
                    for pos, (rc, qc) in enumerate(MM_ORDER):
                        if probe == "half_w" and rc >= 2:
                            continue
                        emit_mm(rc, qc)
                        if pos == RC_LAST[1] and probe != "no_act":
                            emit_tanh(0)
                    if probe not in ("no_act", "half_w"):
                        emit_tanh(1)
            else:  # 'g' mode
                go, gn = gt[t % 2], gt[(t + 1) % 2]
                tn = tt[(t + 1) % 2]

                for pos, (rc, qc) in enumerate(MM_ORDER):
                    half = rc // 2
                    colb = (rc % 2) * 256 + idx * 16
                    nc.tensor.matmul(
                        bank(blk_i, half, par)[:, colb:colb + 16],
                        wt[:, qc * 512 + rc * 128: qc * 512 + (rc + 1) * 128],
                        go[:, qc * 16:(qc + 1) * 16],
                        start=False, stop=(RC_LAST[rc] == pos),
                        skip_group_check=True)
                    if pos == RC_LAST[1] or pos == RC_LAST[3]:
                        half = 0 if pos == RC_LAST[1] else 1
                        b = bank(blk_i, half, par)
                        src = b[:].rearrange("p (r i b) -> p r i b", r=2, i=16)[:, :, idx, :]
                        cols = slice(half * 32, half * 32 + 32)
                        nc.scalar.activation(tn[:, cols], src, AF.Tanh)
                        # g' = c*g + tau   (fused, on chain)
                        nc.vector.scalar_tensor_tensor(
                            gn[:, cols], go[:, cols], c_ap, tn[:, cols],
                            OP.mult, OP.add)

        def finish():
            # final: h = a * (sigma + tau)   [T even -> state in buffer 0]
            fin = timesteps % 2
            g32 = tmpp.tile([128, 64], f32, tag="g32")
            if mode in ("sumap", "2mm"):
                sf = st[fin][:].rearrange("p (q s) -> p q s", q=4)
                g3 = g32[:].rearrange("p (q b) -> p q b", q=4)
                nc.vector.tensor_add(g3, sf[:, :, 0:16], sf[:, :, 16:32])
            else:
                nc.vector.tensor_copy(g32[:], gt[fin][:])
            osb = tmpp.tile([128, 64], f32, tag="osb")
            nc.vector.tensor_scalar_mul(osb[:], g32[:], a_ap)
            nc.gpsimd.dma_start(out_d[:], osb[:])

        if reps == 1:
            body()
        else:
            # large body (>256 instructions/engine): hint the back-edge so
            # the branch target prefetches instead of a ~4us IRAM miss
            ET = mybir.EngineType
            with tc.For_i(0, reps, 1,
                          hint_engines=(ET.PE, ET.Activation, ET.DVE),
                          staggered_reset=stagger):
                body()

    nc.compile()
    return nc


def _host_prep(x, W_in, W_res, lr, wdt=WDT):
    """Build the 8 per-core input maps."""
    import ml_dtypes

    wnp = ml_dtypes.float8_e4m3 if wdt == "f8" else np.float16
    x = np.asarray(x, np.float32)
    W_in = np.asarray(W_in, np.float32)
    W_res = np.asarray(W_res, np.float32)
    lr = np.asarray(lr, np.float32)

    # xt[d, blk*512 + par*256 + i*16 + b] = x[b, blk*32 + 2*i + par, d]
    xr = x.transpose(2, 1, 0)                     # [D, T, B]
    xr = xr.reshape(D, NBLK, BLK // 2, 2, B)      # [d, blk, i, par, b]
    xt = xr.transpose(0, 1, 3, 2, 4).reshape(D, T * 16)
    xt = np.ascontiguousarray(xt, np.float32).astype(np.float16)

    in_maps = []
    for e in range(NCORES):
        a = np.float32(lr[e])
        wtp = (a * W_res[e]).T                    # [q, r]
        wt = np.ascontiguousarray(
            wtp.reshape(4, 128, 512).transpose(1, 0, 2).reshape(128, 2048)
        ).astype(wnp)
        win = np.ascontiguousarray(W_in[e].T).astype(np.float16)  # [8, 512]
        ca = np.empty((128, 2), np.float32)
        ca[:, 0] = 1.0 - a
        ca[:, 1] = a
        in_maps.append({"wt": wt, "win": win, "xt": xt, "ca": ca})
    return in_maps


def _unshard(results):
    out = np.empty((B, E * R), np.float32)
    for e in range(NCORES):
        o = results[e]["out"]                      # [128, 64]
        he = o.reshape(128, 4, 16).transpose(2, 1, 0).reshape(B, R)
        out[:, e * R:(e + 1) * R] = he
    return out


def _run(in_maps, mode=MODE, trace=False, tmpdir=None):
    from concourse import bass_utils

    nc = _build_nc(mode=mode)
    res = bass_utils.run_bass_kernel_spmd(
        nc,
        in_maps,
        core_ids=list(range(NCORES)),
        trace=trace,
        tmpdir=tmpdir,
    )
    return res


_RUNNER = None


def _get_runner():
    """Compile once per process; repeat kernel() calls only re-upload inputs.

    Same lowering as bass2jax.run_bass_via_pjrt's multi-core path, but the
    jitted callable is kept so later calls skip the ~7s rebuild/recompile.
    No output donation: the kernel writes every element of 'out'.
    """
    global _RUNNER
    if _RUNNER is not None:
        return _RUNNER

    import jax
    from jax.sharding import Mesh, NamedSharding, PartitionSpec
    from jax.experimental.shard_map import shard_map
    from concourse import mybir
    from concourse.bass2jax import (
        _bass_exec_p, install_neuronx_cc_hook, partition_id_tensor)

    install_neuronx_cc_hook()
    nc = _build_nc(reps=1)
    partition_name = (
        nc.partition_id_tensor.name if nc.partition_id_tensor else None)
    in_names, out_names, out_avals, zero_outs = [], [], [], []
    for alloc in nc.m.functions[0].allocations:
        if not isinstance(alloc, mybir.MemoryLocationSet):
            continue
        name = alloc.memorylocations[0].name
        if alloc.kind == "ExternalInput":
            if name != partition_name:
                in_names.append(name)
        elif alloc.kind == "ExternalOutput":
            out_avals.append(jax.core.ShapedArray(
                tuple(alloc.tensor_shape), mybir.dt.np(alloc.dtype)))
            out_names.append(name)
            zero_outs.append(np.zeros(
                tuple(alloc.tensor_shape), mybir.dt.np(alloc.dtype)))
    n_params = len(in_names)
    all_in = list(in_names) + list(out_names) + (
        [partition_name] if partition_name else [])

    def _body(*args):
        operands = list(args)
        if partition_name:
            operands.append(partition_id_tensor())
        return tuple(_bass_exec_p.bind(
            *operands, out_avals=tuple(out_avals),
            in_names=tuple(all_in), out_names=tuple(out_names),
            lowering_input_output_aliases=(), sim_require_finite=True,
            sim_require_nnan=True, nc=nc))

    devices = jax.devices()[:NCORES]
    mesh = Mesh(np.asarray(devices), ("core",))
    fn = jax.jit(
        shard_map(
            _body, mesh=mesh,
            in_specs=(PartitionSpec("core"),) * (n_params + len(out_names)),
            out_specs=(PartitionSpec("core"),) * len(out_names),
            check_rep=False),
        keep_unused=True)
    sharding = NamedSharding(mesh, PartitionSpec("core"))
    dev_zeros = [
        jax.device_put(
            np.zeros((NCORES * z.shape[0], *z.shape[1:]), z.dtype), sharding)
        for z in zero_outs
    ]

    def run(in_maps):
        per_core = [[np.asarray(m[n]) for n in in_names] for m in in_maps]
        dev_in = [
            jax.device_put(
                np.concatenate(
                    [per_core[c][i] for c in range(NCORES)], axis=0),
                sharding)
            for i in range(n_params)
        ]
        outs = fn(*dev_in, *dev_zeros)
        return [
            {
                name: np.asarray(outs[i]).reshape(
                    NCORES, *out_avals[i].shape)[c]
                for i, name in enumerate(out_names)
            }
            for c in range(NCORES)
        ]

    _RUNNER = run
    return run


def kernel(x, W_in, W_res, lr):
    in_maps = _host_prep(x, W_in, W_res, lr)
    try:
        results = _get_runner()(in_maps)
    except Exception:
        global _RUNNER
        _RUNNER = None
        results = _run(in_maps, trace=False).results
    return _unshard(results)


if __name__ == "__main__":
    rng = np.random.default_rng(0)
    x = rng.normal(size=(B, T, D)).astype(np.float32)
    W_in = rng.normal(size=(E, R, D)).astype(np.float32) * 0.5
    W_res = (rng.normal(size=(E, R, R)) * (rng.random((E, R, R)) < 0.1)).astype(np.float32) * 0.05
    lr = rng.uniform(0.1, 0.5, E).astype(np.float32)
    out = kernel(x, W_in, W_res, lr)
    print("out", out.shape, out.dtype, np.abs(out).max())

